# revision 20
# baseline (speedup 1.0000x reference)
"""CRF loss (partition - score) Trainium2 kernel — segmented-probe scan.

Problem: B=512, S=1024, T=48 CRF forward algorithm (log-partition via
sequential logsumexp recursion), data-parallel over 8 NeuronCores (64
batch elements per core).

Algorithm (per core, all in probability space):
  - Work with u_t = exp(alpha_t): the per-step logsumexp becomes a matmul
    against E = exp(transitions) plus an elementwise multiply by
    w_t = exp(emissions_t):  a_t[j] = w_t[j] * sum_i E[i,j] a_{t-1}[i].
  - The 1024-step recursion is split into R=28 segments of ~36 steps.
    Products of positive matrices forget their initial direction at a
    geometric rate (Birkhoff contraction), so each segment's chain is
    started W=4 steps early from a probe vector (the staged w at the
    warmup start); by the segment boundary its direction matches the true
    forward chain to well below the tolerance.  Per segment the device
    records the column sums (1^T x) at local steps W-1 and the segment
    end (mixed segment lengths, so two end-record steps); the host
    stitches
      logZ = log n_end(1) + sum_{j=2..R-1} [log n_end(j) - log n_start(j)]
           + log(f . x_R(end)) - log n_start(R)
    (f = exp(end_transitions)), adding back k*c0 per record since E is
    pre-scaled by exp(-c0) (c0 = average per-step log growth, calibrated
    on the host in float64).
  - Two segments are stacked on the 96 partitions of one tile-chain
    (block-diagonal stationary), so 8 tile-chains of N=79 steps cover all
    16 segments.  Chains are grouped 4-wide: one [96x96]x[96,256] matmul
    advances a whole group, and ONE VectorE multiply (FD=256) applies the
    emissions for 4 chains — amortizing the DVE's ~120-cycle fixed PSUM
    read cost, which is the true bottleneck of this recursion (the mult
    must run on DVE: GPSIMD has no PSUM port, ScalarE has no
    tensor*tensor).  Two groups ping-pong so the DVE stays saturated
    while PE/DMA hide underneath.
  - exp() is precomputed on the HOST and staged as bf16 (halving DMA
    bytes and keeping ScalarE out of the pipeline entirely).  State and
    stationaries are bf16; PSUM accumulation is fp32.  No renorms are
    needed: 79-step chains drift only as a tiny random walk around the
    exp(-c0) prescale.

The reference computes `partition - score` where both are the identical
forward algorithm when the mask is all ones (the spec pins mask to ones);
the masked recursion's where(mask, new, old) is the identity then, so
score == partition bitwise.  The kernel computes the shared forward pass
on device and returns their difference.  A faithful numpy fallback
handles a non-all-ones mask, should one ever be passed.
"""

import ml_dtypes
import numpy as np

import concourse.bass as bass
import concourse.bacc as bacc
import concourse.tile as tile
import concourse.mybir as mybir
from concourse.bass_utils import run_bass_kernel_spmd

F32 = mybir.dt.float32
BF16 = mybir.dt.bfloat16
ALU = mybir.AluOpType

N_CORES = 8
B, S, T = 512, 1024, 48
BL = B // N_CORES          # 64 batch elements per core
P2 = 2 * T                 # 96 partitions: two segments stacked per chain

R = 42                     # segments
W = 2                      # warmup steps (probe direction convergence)
N = 28                     # local matmul steps per chain (1..N-1)
# mixed real lengths per segment (sum == S); short segments pad one step
LENS = [27] + [24] * 28 + [25] * 13
NCH = R // 2               # 21 tile-chains
NG = 3                     # chain groups (>=3 hides the mult->matmul->mult
                           # round-trip latency behind the DVE's own period)
GW = NCH // NG             # 7 chains per group
GF = GW * BL               # 448 free-dim columns per group op
# the stationary's extra columns make every matmul also emit the column
# sums m_k = 1^T E' x_{k-1} (rows 96:98) and f-dot (row 98); records are
# plain copies of those rows at these matmul steps:
K_REC = (W, N - 2, N - 1)

# module-level knobs / results (test.py uses these)
TRACE = False
LAST_RESULTS = None

_program_cache = {}


def chunk_plan(n=N):
    """Graded chunk sizes: small first chunks for a fast pipeline ramp."""
    plan, k = [], 0
    for size in [1, 2, 4]:
        size = min(size, n - k)
        if size > 0:
            plan.append((k, size))
            k += size
    while k < n:
        size = min(7, n - k)
        plan.append((k, size))
        k += size
    return plan


def seg_starts():
    """Staged start step s_j per segment (warmup W before each boundary)."""
    b = np.cumsum([0] + LENS)
    return [0] + [int(b[j]) - W for j in range(1, R)]


def build_program(num_devices=N_CORES):
    """Build + compile the per-core Bass/Tile program (SPMD, no collectives)."""
    CW = 96 + 2 + 1            # stationary cols: blockE | sum cols | f col
    SW = NG * len(K_REC) * GF  # sacc cols: (group, record) blocks of GF
    nc = bacc.Bacc(
        "TRN2",
        target_bir_lowering=False,
        debug=False,
        num_devices=num_devices,
    )
    # wstg step 0 carries the stationary (consts padded to a full step row);
    # scan step k lives at wstg step k+1
    wstg = nc.dram_tensor("wstg", [P2, N + 1, NG * GF], BF16, kind="ExternalInput").ap()
    out_s = nc.dram_tensor("sacc", [3, SW], BF16, kind="ExternalOutput").ap()

    SW_STEP = NG * GF          # cols per step in wstg

    with tile.TileContext(nc) as tc:
        with (
            tc.tile_pool(name="head", bufs=1) as cpool,
            tc.tile_pool(name="raw", bufs=3) as rawpool,
            tc.tile_pool(name="state", bufs=2) as xpool,
            tc.tile_pool(name="sacc_p", bufs=1) as sapool,
            tc.tile_pool(name="psum_v", bufs=2, space=bass.MemorySpace.PSUM) as ppool,
        ):
            # one persistent head tile holds the stationary AND scan step 0;
            # a single DMA starts the whole pipeline.
            head = cpool.tile([P2, 2 * SW_STEP], BF16)
            nc.sync.dma_start(
                head[:], wstg[:, 0:2, :].rearrange("p k b -> p (k b)"))
            # one stationary for the whole kernel: block-diag E' plus the
            # sum columns (96,97) and the f-dot column (98); every matmul
            # computes the scan step AND the records in one pass, so the
            # weights are loaded exactly once (ldweights=False after that).
            lhsT = head[:, 0:CW]

            # record rows live on partitions 96..98 (engine lanes are fixed:
            # the copy from v[96:99] must write the same partitions)
            sacc_full = sapool.tile([CW, SW], BF16)
            sacc = sacc_full[96:CW, :]

            first_mm = True
            xs = [None] * NG
            w0 = [head[:, SW_STEP + g * GF:SW_STEP + (g + 1) * GF]
                  for g in range(NG)]
            for (k0, klen) in chunk_plan(N - 1):
                # chunk covers scan steps k0+1 .. k0+klen (wstg steps +1 more)
                raw = rawpool.tile([P2, klen * SW_STEP], BF16, tag="raw", name="raw")
                nc.sync.dma_start(
                    raw[:], wstg[:, k0 + 2:k0 + 2 + klen, :].rearrange("p k b -> p (k b)"))
                for kl in range(klen):
                    k = k0 + 1 + kl
                    for g in range(NG):
                        wk = raw[:, kl * SW_STEP + g * GF:kl * SW_STEP + (g + 1) * GF]
                        v = ppool.tile([CW, GF], F32, tag=f"v{g}")
                        mov = xs[g][:] if k > 1 else w0[g]
                        mm = nc.tensor.matmul(v[:], lhsT, mov, start=True, stop=True)
                        if first_mm:
                            first_mm = False
                        else:
                            mm.ins.ldweights = False
                        if k in K_REC:
                            ri = K_REC.index(k)
                            col = (g * len(K_REC) + ri) * GF
                            nc.scalar.copy(sacc[:, col:col + GF], v[96:CW, :])
                        if k == N - 1:
                            continue  # nothing consumes x_{N-1}
                        xs[g] = xpool.tile([P2, GF], BF16, tag=f"x{g}", name=f"x{g}")
                        # (v * 1.0) * w via the TensorScalarPtr op family —
                        # measured faster than tensor_tensor for this shape
                        nc.vector.scalar_tensor_tensor(
                            xs[g][:], v[0:P2, :], 1.0, wk, ALU.mult, ALU.mult)

            nc.sync.dma_start(out_s, sacc[:])

    nc.compile()
    return nc


def _get_program():
    key = "full"
    if key not in _program_cache:
        _program_cache[key] = build_program()
    return _program_cache[key]


def _calibrate_c0(emissions, start, trans, n_batches=8):
    """Average per-step log growth of the forward recursion (float64)."""
    idx = np.linspace(0, emissions.shape[0] - 1, n_batches).astype(np.int64)
    E = np.exp(trans.astype(np.float64))
    u = np.exp(start.astype(np.float64))[None, :] * \
        np.exp(emissions[idx, 0].astype(np.float64))
    s = u.sum(axis=1, keepdims=True)
    u /= s
    tot = 0.0
    n = emissions.shape[1]
    for t in range(1, n):
        u = np.exp(emissions[idx, t].astype(np.float64)) * (u @ E)
        s = u.sum(axis=1, keepdims=True)
        u /= s
        tot += np.log(s).mean()
    return tot / (n - 1)


def make_consts(Ep_bf16, end):
    CW = 96 + 2 + 1
    consts = np.zeros((P2, CW), ml_dtypes.bfloat16)
    consts[:T, :T] = Ep_bf16                   # half-A forward block
    consts[T:, T:2 * T] = Ep_bf16              # half-B forward block
    consts[:T, 96] = 1.0                       # lhsT_sum col 0: half-A sum
    consts[T:, 97] = 1.0                       # lhsT_sum col 1: half-B sum
    consts[T:, 98] = np.exp(end.astype(np.float64)).astype(ml_dtypes.bfloat16)
    return consts


def stage_inputs(emissions, start, end, trans):
    """Host-side restaging: per-core [P2, N, 512] bf16 exp(emission) tiles."""
    c0 = _calibrate_c0(emissions, start, trans)
    Ep = (np.exp(trans.astype(np.float64)) * np.exp(-c0)).astype(ml_dtypes.bfloat16)
    consts = make_consts(Ep, end)

    in_maps = []
    for core in range(N_CORES):
        sl = slice(core * BL, (core + 1) * BL)
        em = emissions[sl].astype(np.float32).copy()      # [BL, S, T]
        em[:, 0, :] += start.astype(np.float32)
        w = np.exp(em).astype(ml_dtypes.bfloat16)          # [BL, S, T]
        starts = seg_starts()
        # staged step indices, clipped at S-1 (short segments pad one step
        # past their end-record; the padded state is never recorded)
        tidx = np.minimum(np.arange(N)[None, :] + np.array(starts)[:, None], S - 1)
        stg = np.zeros((P2, N + 1, NG * GF), ml_dtypes.bfloat16)
        stg[:, 0, :consts.shape[1]] = consts   # stationary rides as step 0
        for c in range(NCH):
            g, cg = divmod(c, GW)
            colsl = slice(g * GF + cg * BL, g * GF + (cg + 1) * BL)
            stg[:T, 1:, colsl] = w[:, tidx[2 * c], :].transpose(2, 1, 0)
            stg[T:, 1:, colsl] = w[:, tidx[2 * c + 1], :].transpose(2, 1, 0)
        in_maps.append({"wstg": stg})
    return in_maps, c0


def unpack_logZ(sacc, c0):
    """Recover logZ[BL] from device outputs of one core (float64 host math)."""
    lsac = np.log(sacc.astype(np.float64))     # [3, NG*len(K_REC)*GF]
    # record at matmul step k captures m_k = 1^T E' x_{k-1}: the state has
    # k-1 E' applications plus one inside m -> add k*c0.  The f-dot row
    # has no E' of its own -> add (k-1)*c0.
    Lrec = np.empty((2, NCH, len(K_REC), BL))  # [half, chain, rec, batch]
    for c in range(NCH):
        g, cg = divmod(c, GW)
        for ri, k in enumerate(K_REC):
            col = (g * len(K_REC) + ri) * GF + cg * BL
            Lrec[:, c, ri] = lsac[:2, col:col + BL] + k * c0
    # f-dot of the final state: last record (k=N-1), row 2, last chain half B
    col = ((NG - 1) * len(K_REC) + (len(K_REC) - 1)) * GF + (GW - 1) * BL
    Lfdot = lsac[2, col:col + BL] + (N - 2) * c0
    # segment j = 2c + h; its end-record is at matmul step W+LENS[j]
    # (j=0: LENS[0], exact from t=0); j=R-1 ends with the f-dot
    Lst = np.empty((R, BL))
    Len = np.empty((R, BL))
    for c in range(NCH):
        for h in range(2):
            j = 2 * c + h
            ke = (W + LENS[j]) if j > 0 else LENS[0]
            Lst[j] = Lrec[h, c, 0]
            Len[j] = Lrec[h, c, K_REC.index(ke)]
    logZ = Len[0].copy()
    for j in range(1, R - 1):
        logZ += Len[j] - Lst[j]
    logZ += Lfdot - Lst[R - 1]
    return logZ


def _device_logZ(emissions, start, end, trans):
    global LAST_RESULTS
    nc = _get_program()
    in_maps, c0 = stage_inputs(emissions, start, end, trans)
    res = run_bass_kernel_spmd(
        nc, in_maps, core_ids=list(range(N_CORES)), trace=TRACE,
    )
    LAST_RESULTS = res
    logZ = np.empty(B, np.float32)
    for core in range(N_CORES):
        r = res.results[core]
        logZ[core * BL:(core + 1) * BL] = unpack_logZ(
            np.asarray(r["sacc"]), c0).astype(np.float32)
    return logZ


def _numpy_fallback(emissions, mask, start, end, trans):
    """Faithful float64 reference implementation (handles any mask)."""
    def fwd(use_mask):
        a = start[None, :].astype(np.float64) + emissions[:, 0].astype(np.float64)
        tr = trans.astype(np.float64)
        for t in range(1, emissions.shape[1]):
            inner = a[:, :, None] + tr[None] + emissions[:, t].astype(np.float64)[:, None, :]
            m = inner.max(axis=1, keepdims=True)
            new = np.log(np.exp(inner - m).sum(axis=1)) + m[:, 0, :]
            if use_mask:
                a = np.where(mask[:, t][:, None], new, a)
            else:
                a = new
        fin = a + end[None].astype(np.float64)
        m = fin.max(axis=1, keepdims=True)
        return np.log(np.exp(fin - m).sum(axis=1)) + m[:, 0]

    score = fwd(True)
    partition = fwd(False)
    return (partition - score).astype(np.float32)


def kernel(emissions, mask, start_transitions, end_transitions, transitions):
    emissions = np.asarray(emissions, dtype=np.float32)
    mask = np.asarray(mask)
    start = np.asarray(start_transitions, dtype=np.float32)
    end = np.asarray(end_transitions, dtype=np.float32)
    trans = np.asarray(transitions, dtype=np.float32)

    if not mask.all():
        return _numpy_fallback(emissions, mask, start, end, trans)

    # With an all-ones mask the masked recursion's where(mask, new, old) is
    # the identity, so score == partition; both come from the same forward
    # pass, computed on the 8 NeuronCores.
    logZ = _device_logZ(emissions, start, end, trans)
    partition = logZ
    score = logZ
    return (partition - score).astype(np.float32)
